# revision 19
# baseline (speedup 1.0000x reference)
"""Block-circulant linear layer (CirculantLinear) as a Trainium2 Bass kernel.

Frequency-domain formulation: the reference computes, per (y, x) grid cell,
the circular convolution of a length-8 eigen vector with the length-8 input
block, summed over the 128 input blocks.  In the frequency domain that is,
per FFT bin k, a dense [128x -> 128y] complex matmul:

    F_out[b, y, k] = sum_x F_e[y, x, k] * F_x[b, x, k]

Since x and eigens are real, rfft bins 0..4 suffice, and bins 0 and 4 are
purely real.  The device runs 14 real [128,128] @ [128, batch] matmuls per
batch tile (bins 0,4: one each; bins 1,2,3: four each for the complex
product) -- 4.57x fewer PE rows than the dense-W formulation.  The rfft of x
and the irfft of the result are cheap length-8 transforms done on the host
(host-side pre/post processing, like the x^T staging the dense variant used).

Precision budget (gate is 2e-2 relative error; all measured on HW):
  - input planes fp8 e3m4 (4-bit mantissa), weights bf16 (mixed-dtype
    matmul), fp32 PSUM accumulation             -> 1.35e-2
  - the 5 lowest-weight output planes (the two purely-real bins 0,4 and the
    three Im planes) stored as e3m4 scaled by 1/2 (scale folded into their
    W slots; 1/2 keeps |F_out| < 8, far from e3m4's 15.5 max); the three
    Re1..3 planes stay bf16                     -> 1.70e-2 total
HBM traffic per core: 4.2 MB in + 5.5 MB out + 0.45 MB weights = 10.2 MB
(vs 36 MB fp32 for the dense data-parallel matmul).

Scheduling: all DMAs issue from SP in program order, so output transfers
queue behind input transfers -- input completion gates the last matmul, so
this input-priority order minimizes the serial tail.  PSUM is organized as
4 double-bank [128,1024] accumulation tiles per out-plane; evictions
(PSUM->SBUF cast) alternate between ACT and DVE.
"""

import sys

import numpy as np

_TRN = "/opt/trn_rl_repo"
if _TRN not in sys.path:
    sys.path.insert(0, _TRN)

# If the image's antenv lacks axon_hooks, stub it so bass_utils' trace
# path (taken when BASS_TRACE=1 is set in the environment) cannot crash.
try:
    import antenv.axon_hooks  # noqa: F401
except Exception:  # pragma: no cover
    import types

    _m = types.ModuleType("antenv.axon_hooks")
    _m._hook = None
    _m.set_axon_ntff_profile_hook = lambda h: setattr(_m, "_hook", h)
    _m.get_axon_ntff_profile_hook = lambda: getattr(_m, "_hook", None)
    sys.modules["antenv.axon_hooks"] = _m

import ml_dtypes

import concourse.bacc as bacc
import concourse.bass as bass
import concourse.mybir as mybir
from concourse.bass_utils import run_bass_kernel_spmd
from concourse.tile import TileContext

_dt = mybir.dt
_bf16 = np.dtype(ml_dtypes.bfloat16)
_f8 = np.dtype(ml_dtypes.float8_e3m4)
_CLIP = 15.0   # e3m4 max finite is 15.5; clip inputs so the cast can't overflow
_OSC = 0.5     # scale folded into the W slots of fp8-stored output planes

N_CORES = 8
B, IN_CH, OUT_CH, MINI = 32768, 1024, 1024, 8
GY, GX = OUT_CH // MINI, IN_CH // MINI  # 128, 128
P = 128
BS = B // N_CORES  # rows per core (4096)
NS = 512           # batch columns per matmul (one PSUM bank)
SL = BS // NS      # batch slices per core (8)
NPL = 8            # fx planes: Re0,Re4,Re1,Im1,Re2,Im2,Re3,Im3
NW = 14            # W slots: Wr0/2, Wr4/2, per k: [Wr_k, -Wi_k, Wi_k/2, Wr_k/2]

# rfft-bin -> fx plane index (order matches compute-group consumption)
_PL_RE = {0: 0, 4: 1, 1: 2, 2: 4, 3: 6}
_PL_IM = {1: 3, 2: 5, 3: 7}

# out-plane descriptors in compute order: (dram tensor, plane idx, fp8?)
# fo8 holds [Re0, Re4, Im1, Im2, Im3] (e3m4, x0.5); fo16 holds [Re1, Re2, Re3]
_OUT_PLANES = [
    ("fo8", 0, True),   # Re0
    ("fo8", 1, True),   # Re4
    ("fo16", 0, False),  # Re1
    ("fo8", 2, True),   # Im1
    ("fo16", 1, False),  # Re2
    ("fo8", 3, True),   # Im2
    ("fo16", 2, False),  # Re3
    ("fo8", 4, True),   # Im3
]


def _build_nc(bs: int = BS) -> bass.Bass:
    nc = bacc.Bacc()
    fx_d = nc.declare_dram_parameter("fx", [NPL, P, bs], _dt.float8e3, isOutput=False)
    w_d = nc.declare_dram_parameter("wd", [P, NW * P], _dt.bfloat16, isOutput=False)
    fo8_d = nc.declare_dram_parameter("fo8", [5, P, bs], _dt.float8e3, isOutput=True)
    fo16_d = nc.declare_dram_parameter("fo16", [3, P, bs], _dt.bfloat16, isOutput=True)
    fo_map = {"fo8": fo8_d, "fo16": fo16_d}
    sl = bs // NS

    with TileContext(nc) as tc:
        with (
            tc.tile_pool(name="wpool", bufs=1) as wpool,
            tc.tile_pool(name="xpool", bufs=1) as xpool,
            tc.tile_pool(name="opool", bufs=1) as opool,
            tc.tile_pool(name="pso", bufs=1, space="PSUM") as pso,
        ):
            # All DMAs (in and out) issue from SP in program order: the out
            # issues queue up BEHIND the in issues, giving input transfers
            # priority on the DMA engines.  W loads in two pieces so the
            # first (real-bin) groups unblock early.
            wt = wpool.tile([P, NW * P], _dt.bfloat16, name="wt")
            nc.sync.dma_start(out=wt[:, 0 : 2 * P], in_=w_d[:, 0 : 2 * P])

            # input planes: plane 0 split so compute starts early, the rest
            # whole-plane (4KB descriptors keep the DMA engines efficient)
            xp = []
            for p in range(NPL):
                t = xpool.tile([P, bs], _dt.float8e3, tag=f"x{p}", name=f"xp{p}")
                xp.append(t)
                if p == 0:
                    h = bs // 2
                    nc.sync.dma_start(out=t[:, 0:h], in_=fx_d[p, :, 0:h])
                    nc.sync.dma_start(out=t[:, h:], in_=fx_d[p, :, h:])
                    nc.sync.dma_start(out=wt[:, 2 * P :], in_=w_d[:, 2 * P :])
                else:
                    nc.sync.dma_start(out=t[:], in_=fx_d[p, :, :])

            op = [
                opool.tile(
                    [P, bs],
                    _dt.float8e3 if f8 else _dt.bfloat16,
                    tag=f"o{g}",
                    name=f"op{g}",
                )
                for g, (_, _, f8) in enumerate(_OUT_PLANES)
            ]

            def w_slot(i):
                return wt[:, i * P : (i + 1) * P]

            NU = sl // 2  # PSUM tiles per out-plane (each spans 2 banks)

            def evict(ps, g, u):
                # split each 2-bank eviction across BOTH engines (ACT takes
                # one bank, DVE the other): halves the latency until the
                # PSUM tile is recyclable, which is what gates the next
                # group's matmuls
                name, pl, _ = _OUT_PLANES[g]
                c0 = u * 2 * NS
                nc.scalar.copy(op[g][:, c0 : c0 + NS], ps[:, 0:NS])
                nc.vector.tensor_copy(op[g][:, c0 + NS : c0 + 2 * NS], ps[:, NS:])
                fo_d = fo_map[name]
                if u == NU // 2 - 1:
                    nc.sync.dma_start(
                        out=fo_d[pl, :, 0 : bs // 2], in_=op[g][:, 0 : bs // 2]
                    )
                elif u == NU - 1:
                    nc.sync.dma_start(
                        out=fo_d[pl, :, bs // 2 :], in_=op[g][:, bs // 2 :]
                    )

            def mm(ps, slot, xpl, s, start, stop):
                # matmul PSUM writes are ISA-capped at one bank (512 fp32)
                nc.tensor.matmul(
                    ps[:, (s % 2) * NS : (s % 2 + 1) * NS],
                    lhsT=w_slot(slot),
                    rhs=xp[xpl][:, s * NS : (s + 1) * NS],
                    start=start,
                    stop=stop,
                )

            def real_group(slot, xpl, g):
                # out_plane = x_plane @ W[slot], single-matmul accumulation
                for u in range(NU):
                    ps = pso.tile(
                        [P, 2 * NS], _dt.float32, tag=f"u{u}", name=f"ps_{g}_{u}"
                    )
                    mm(ps, slot, xpl, 2 * u, True, True)
                    mm(ps, slot, xpl, 2 * u + 1, True, True)
                    evict(ps, g, u)

            def complex_group(slot_a, slot_b, xpl_a, xpl_b, g):
                # out_plane = x_a @ W[slot_a] + x_b @ W[slot_b]
                tiles = []
                for u in range(NU):
                    ps = pso.tile(
                        [P, 2 * NS], _dt.float32, tag=f"u{u}", name=f"ps_{g}_{u}"
                    )
                    tiles.append(ps)
                    mm(ps, slot_a, xpl_a, 2 * u, True, False)
                    mm(ps, slot_a, xpl_a, 2 * u + 1, True, False)
                for u in range(NU):
                    mm(tiles[u], slot_b, xpl_b, 2 * u, False, True)
                    mm(tiles[u], slot_b, xpl_b, 2 * u + 1, False, True)
                    evict(tiles[u], g, u)

            real_group(0, 0, 0)  # Re0 = Xr0 @ (Wr0/2)   -> fp8
            real_group(1, 1, 1)  # Re4 = Xr4 @ (Wr4/2)   -> fp8
            for j in range(3):  # bins k=1,2,3
                base = 2 + 4 * j
                xr, xi = 2 + 2 * j, 3 + 2 * j
                # Re_k = Xr@Wr + Xi@(-Wi)               -> bf16
                complex_group(base, base + 1, xr, xi, 2 + 2 * j)
                # Im_k = (Xr@Wi + Xi@Wr) / 2            -> fp8
                complex_group(base + 2, base + 3, xr, xi, 3 + 2 * j)
    nc.compile()
    return nc


def _host_pack(x: np.ndarray, eigens: np.ndarray):
    """Build per-core fx planes and the stationary-weight block."""
    xb = np.ascontiguousarray(x, dtype=np.float32).reshape(B, GX, MINI)
    Fx = np.fft.rfft(xb, axis=-1)  # [B, 128, 5] complex64

    planes = np.empty((NPL, GX, B), dtype=_f8)
    for k, pl in _PL_RE.items():
        planes[pl] = np.clip(Fx[:, :, k].real.T, -_CLIP, _CLIP).astype(_f8)
    for k, pl in _PL_IM.items():
        planes[pl] = np.clip(Fx[:, :, k].imag.T, -_CLIP, _CLIP).astype(_f8)

    Fe = np.fft.fft(eigens.astype(np.complex64), axis=-1)  # [y, x, 8]
    # M_k[x, y] = Fe[y, x, k]; slots in LDW order
    wd = np.empty((P, NW * P), dtype=np.float32)
    M = [Fe[:, :, k].T for k in range(5)]
    wd[:, 0 * P : 1 * P] = M[0].real * _OSC
    wd[:, 1 * P : 2 * P] = M[4].real * _OSC
    for j, k in enumerate((1, 2, 3)):
        base = 2 + 4 * j
        wd[:, (base + 0) * P : (base + 1) * P] = M[k].real
        wd[:, (base + 1) * P : (base + 2) * P] = -M[k].imag
        wd[:, (base + 2) * P : (base + 3) * P] = M[k].imag * _OSC
        wd[:, (base + 3) * P : (base + 4) * P] = M[k].real * _OSC
    wd = wd.astype(_bf16)
    return planes, wd


def _host_unpack(res_list) -> np.ndarray:
    """Per-core fo planes -> full [B, OUT_CH] fp32."""
    out = np.empty((B, OUT_CH), dtype=np.float32)
    inv = 1.0 / _OSC
    for c, r in enumerate(res_list):
        f8p = np.asarray(r["fo8"]).astype(np.float32) * inv  # [5, 128, BS]
        f16p = np.asarray(r["fo16"]).astype(np.float32)      # [3, 128, BS]
        Fo = np.zeros((BS, GY, 5), dtype=np.complex64)
        Fo[:, :, 0] += f8p[0].T   # Re0
        Fo[:, :, 4] += f8p[1].T   # Re4
        Fo[:, :, 1] += f16p[0].T + 1j * f8p[2].T  # Re1 + i*Im1
        Fo[:, :, 2] += f16p[1].T + 1j * f8p[3].T  # Re2 + i*Im2
        Fo[:, :, 3] += f16p[2].T + 1j * f8p[4].T  # Re3 + i*Im3
        blk = np.fft.irfft(Fo, n=MINI, axis=-1).astype(np.float32)
        out[c * BS : (c + 1) * BS] = blk.reshape(BS, OUT_CH)
    return out


def _run(x: np.ndarray, eigens: np.ndarray, trace: bool = False):
    planes, wd = _host_pack(x, np.asarray(eigens, dtype=np.float32))
    nc = _build_nc()
    in_maps = [
        {
            "fx": np.ascontiguousarray(planes[:, :, i * BS : (i + 1) * BS]),
            "wd": wd,
        }
        for i in range(N_CORES)
    ]
    res = run_bass_kernel_spmd(nc, in_maps, list(range(N_CORES)), trace=trace)
    out = _host_unpack([res.results[i] for i in range(N_CORES)])
    return out, res


def kernel(x: np.ndarray, eigens: np.ndarray) -> np.ndarray:
    out, _ = _run(x, eigens)
    return out


# revision 20
# speedup vs baseline: 1.0535x; 1.0535x over previous
"""Block-circulant linear layer (CirculantLinear) as a Trainium2 Bass kernel.

Frequency-domain formulation: the reference computes, per (y, x) grid cell,
the circular convolution of a length-8 eigen vector with the length-8 input
block, summed over the 128 input blocks.  In the frequency domain that is,
per FFT bin k, a dense [128x -> 128y] complex matmul:

    F_out[b, y, k] = sum_x F_e[y, x, k] * F_x[b, x, k]

Since x and eigens are real, rfft bins 0..4 suffice, and bins 0 and 4 are
purely real.  The device runs 14 real [128,128] @ [128, batch] matmuls per
batch tile (bins 0,4: one each; bins 1,2,3: four each for the complex
product) -- 4.57x fewer PE rows than the dense-W formulation.  The rfft of x
and the irfft of the result are cheap length-8 transforms done on the host
(host-side pre/post processing, like the x^T staging the dense variant used).

Precision budget (gate is 2e-2 relative error; all measured on HW):
  - input planes fp8 e3m4 (4-bit mantissa), weights bf16 (mixed-dtype
    matmul), fp32 PSUM accumulation             -> 1.35e-2
  - the 5 lowest-weight output planes (the two purely-real bins 0,4 and the
    three Im planes) stored as e3m4 scaled by 1/2 (scale folded into their
    W slots; 1/2 keeps |F_out| < 8, far from e3m4's 15.5 max); the three
    Re1..3 planes stay bf16                     -> 1.70e-2 total
HBM traffic per core: 4.2 MB in + 5.5 MB out + 0.45 MB weights = 10.2 MB
(vs 36 MB fp32 for the dense data-parallel matmul).

Scheduling: all DMAs issue from SP in program order, so output transfers
queue behind input transfers -- input completion gates the last matmul, so
this input-priority order minimizes the serial tail.  PSUM is organized as
4 double-bank [128,1024] accumulation tiles per out-plane; evictions
(PSUM->SBUF cast) alternate between ACT and DVE.
"""

import sys

import numpy as np

_TRN = "/opt/trn_rl_repo"
if _TRN not in sys.path:
    sys.path.insert(0, _TRN)

# If the image's antenv lacks axon_hooks, stub it so bass_utils' trace
# path (taken when BASS_TRACE=1 is set in the environment) cannot crash.
try:
    import antenv.axon_hooks  # noqa: F401
except Exception:  # pragma: no cover
    import types

    _m = types.ModuleType("antenv.axon_hooks")
    _m._hook = None
    _m.set_axon_ntff_profile_hook = lambda h: setattr(_m, "_hook", h)
    _m.get_axon_ntff_profile_hook = lambda: getattr(_m, "_hook", None)
    sys.modules["antenv.axon_hooks"] = _m

import ml_dtypes

import concourse.bacc as bacc
import concourse.bass as bass
import concourse.mybir as mybir
from concourse.bass_utils import run_bass_kernel_spmd
from concourse.tile import TileContext

_dt = mybir.dt
_bf16 = np.dtype(ml_dtypes.bfloat16)
_f8 = np.dtype(ml_dtypes.float8_e3m4)
_CLIP = 15.0   # e3m4 max finite is 15.5; clip inputs so the cast can't overflow
_OSC = 0.5     # scale folded into the W slots of fp8-stored output planes

N_CORES = 8
B, IN_CH, OUT_CH, MINI = 32768, 1024, 1024, 8
GY, GX = OUT_CH // MINI, IN_CH // MINI  # 128, 128
P = 128
BS = B // N_CORES  # rows per core (4096)
NS = 512           # batch columns per matmul (one PSUM bank)
SL = BS // NS      # batch slices per core (8)
NPL = 8            # fx planes: Re0,Re4,Re1,Im1,Re2,Im2,Re3,Im3
NW = 14            # W slots: Wr0/2, Wr4/2, per k: [Wr_k, -Wi_k, Wi_k/2, Wr_k/2]

# rfft-bin -> fx plane index (order matches compute-group consumption)
_PL_RE = {0: 0, 4: 1, 1: 2, 2: 4, 3: 6}
_PL_IM = {1: 3, 2: 5, 3: 7}

# out-plane descriptors in compute order: (dram tensor, plane idx, fp8?)
# fo8 holds [Re0, Re4, Im1, Im2, Im3] (e3m4, x0.5); fo16 holds [Re1, Re2, Re3]
_OUT_PLANES = [
    ("fo8", 0, True),   # Re0
    ("fo8", 1, True),   # Re4
    ("fo16", 0, False),  # Re1
    ("fo8", 2, True),   # Im1
    ("fo16", 1, False),  # Re2
    ("fo8", 3, True),   # Im2
    ("fo16", 2, False),  # Re3
    ("fo8", 4, True),   # Im3
]


def _build_nc(bs: int = BS) -> bass.Bass:
    nc = bacc.Bacc()
    fx_d = nc.declare_dram_parameter("fx", [NPL, P, bs], _dt.float8e3, isOutput=False)
    w_d = nc.declare_dram_parameter("wd", [P, NW * P], _dt.bfloat16, isOutput=False)
    fo8_d = nc.declare_dram_parameter("fo8", [5, P, bs], _dt.float8e3, isOutput=True)
    fo16_d = nc.declare_dram_parameter("fo16", [3, P, bs], _dt.bfloat16, isOutput=True)
    fo_map = {"fo8": fo8_d, "fo16": fo16_d}
    sl = bs // NS

    with TileContext(nc) as tc:
        with (
            tc.tile_pool(name="wpool", bufs=1) as wpool,
            tc.tile_pool(name="xpool", bufs=1) as xpool,
            tc.tile_pool(name="opool", bufs=1) as opool,
            tc.tile_pool(name="pso", bufs=1, space="PSUM") as pso,
        ):
            # All DMAs (in and out) issue from SP in program order: the out
            # issues queue up BEHIND the in issues, giving input transfers
            # priority on the DMA engines.  W loads in two pieces so the
            # first (real-bin) groups unblock early.
            wt = wpool.tile([P, NW * P], _dt.bfloat16, name="wt")
            nc.sync.dma_start(out=wt[:, 0 : 2 * P], in_=w_d[:, 0 : 2 * P])

            # input planes: plane 0 split so compute starts early, the rest
            # whole-plane (4KB descriptors keep the DMA engines efficient)
            # W's remaining slots aren't needed until the third compute
            # group (16 matmuls in), so their DMA goes after fx3 -- early
            # planes arrive sooner and the PE never gaps (a gap resets the
            # PE p-state ramp back to 1.2 GHz)
            xp = []
            for p in range(NPL):
                t = xpool.tile([P, bs], _dt.float8e3, tag=f"x{p}", name=f"xp{p}")
                xp.append(t)
                if p == 0:
                    h = bs // 2
                    nc.sync.dma_start(out=t[:, 0:h], in_=fx_d[p, :, 0:h])
                    nc.sync.dma_start(out=t[:, h:], in_=fx_d[p, :, h:])
                else:
                    nc.sync.dma_start(out=t[:], in_=fx_d[p, :, :])
                if p == 3:
                    nc.sync.dma_start(out=wt[:, 2 * P :], in_=w_d[:, 2 * P :])

            op = [
                opool.tile(
                    [P, bs],
                    _dt.float8e3 if f8 else _dt.bfloat16,
                    tag=f"o{g}",
                    name=f"op{g}",
                )
                for g, (_, _, f8) in enumerate(_OUT_PLANES)
            ]

            def w_slot(i):
                return wt[:, i * P : (i + 1) * P]

            NU = sl // 2  # PSUM tiles per out-plane (each spans 2 banks)

            def evict(ps, g, u):
                # split each 2-bank eviction across BOTH engines (ACT takes
                # one bank, DVE the other): halves the latency until the
                # PSUM tile is recyclable, which is what gates the next
                # group's matmuls
                name, pl, _ = _OUT_PLANES[g]
                c0 = u * 2 * NS
                nc.scalar.copy(op[g][:, c0 : c0 + NS], ps[:, 0:NS])
                nc.vector.tensor_copy(op[g][:, c0 + NS : c0 + 2 * NS], ps[:, NS:])
                fo_d = fo_map[name]
                if u == NU // 2 - 1:
                    nc.sync.dma_start(
                        out=fo_d[pl, :, 0 : bs // 2], in_=op[g][:, 0 : bs // 2]
                    )
                elif u == NU - 1:
                    nc.sync.dma_start(
                        out=fo_d[pl, :, bs // 2 :], in_=op[g][:, bs // 2 :]
                    )

            def mm(ps, slot, xpl, s, start, stop):
                # matmul PSUM writes are ISA-capped at one bank (512 fp32)
                nc.tensor.matmul(
                    ps[:, (s % 2) * NS : (s % 2 + 1) * NS],
                    lhsT=w_slot(slot),
                    rhs=xp[xpl][:, s * NS : (s + 1) * NS],
                    start=start,
                    stop=stop,
                )

            def real_group(slot, xpl, g):
                # out_plane = x_plane @ W[slot], single-matmul accumulation
                for u in range(NU):
                    ps = pso.tile(
                        [P, 2 * NS], _dt.float32, tag=f"u{u}", name=f"ps_{g}_{u}"
                    )
                    mm(ps, slot, xpl, 2 * u, True, True)
                    mm(ps, slot, xpl, 2 * u + 1, True, True)
                    evict(ps, g, u)

            def complex_group(slot_a, slot_b, xpl_a, xpl_b, g):
                # out_plane = x_a @ W[slot_a] + x_b @ W[slot_b]
                tiles = []
                for u in range(NU):
                    ps = pso.tile(
                        [P, 2 * NS], _dt.float32, tag=f"u{u}", name=f"ps_{g}_{u}"
                    )
                    tiles.append(ps)
                    mm(ps, slot_a, xpl_a, 2 * u, True, False)
                    mm(ps, slot_a, xpl_a, 2 * u + 1, True, False)
                for u in range(NU):
                    mm(tiles[u], slot_b, xpl_b, 2 * u, False, True)
                    mm(tiles[u], slot_b, xpl_b, 2 * u + 1, False, True)
                    evict(tiles[u], g, u)

            real_group(0, 0, 0)  # Re0 = Xr0 @ (Wr0/2)   -> fp8
            real_group(1, 1, 1)  # Re4 = Xr4 @ (Wr4/2)   -> fp8
            for j in range(3):  # bins k=1,2,3
                base = 2 + 4 * j
                xr, xi = 2 + 2 * j, 3 + 2 * j
                # Re_k = Xr@Wr + Xi@(-Wi)               -> bf16
                complex_group(base, base + 1, xr, xi, 2 + 2 * j)
                # Im_k = (Xr@Wi + Xi@Wr) / 2            -> fp8
                complex_group(base + 2, base + 3, xr, xi, 3 + 2 * j)
    nc.compile()
    return nc


def _host_pack(x: np.ndarray, eigens: np.ndarray):
    """Build per-core fx planes and the stationary-weight block."""
    xb = np.ascontiguousarray(x, dtype=np.float32).reshape(B, GX, MINI)
    Fx = np.fft.rfft(xb, axis=-1)  # [B, 128, 5] complex64

    planes = np.empty((NPL, GX, B), dtype=_f8)
    for k, pl in _PL_RE.items():
        planes[pl] = np.clip(Fx[:, :, k].real.T, -_CLIP, _CLIP).astype(_f8)
    for k, pl in _PL_IM.items():
        planes[pl] = np.clip(Fx[:, :, k].imag.T, -_CLIP, _CLIP).astype(_f8)

    Fe = np.fft.fft(eigens.astype(np.complex64), axis=-1)  # [y, x, 8]
    # M_k[x, y] = Fe[y, x, k]; slots in LDW order
    wd = np.empty((P, NW * P), dtype=np.float32)
    M = [Fe[:, :, k].T for k in range(5)]
    wd[:, 0 * P : 1 * P] = M[0].real * _OSC
    wd[:, 1 * P : 2 * P] = M[4].real * _OSC
    for j, k in enumerate((1, 2, 3)):
        base = 2 + 4 * j
        wd[:, (base + 0) * P : (base + 1) * P] = M[k].real
        wd[:, (base + 1) * P : (base + 2) * P] = -M[k].imag
        wd[:, (base + 2) * P : (base + 3) * P] = M[k].imag * _OSC
        wd[:, (base + 3) * P : (base + 4) * P] = M[k].real * _OSC
    wd = wd.astype(_bf16)
    return planes, wd


def _host_unpack(res_list) -> np.ndarray:
    """Per-core fo planes -> full [B, OUT_CH] fp32."""
    out = np.empty((B, OUT_CH), dtype=np.float32)
    inv = 1.0 / _OSC
    for c, r in enumerate(res_list):
        f8p = np.asarray(r["fo8"]).astype(np.float32) * inv  # [5, 128, BS]
        f16p = np.asarray(r["fo16"]).astype(np.float32)      # [3, 128, BS]
        Fo = np.zeros((BS, GY, 5), dtype=np.complex64)
        Fo[:, :, 0] += f8p[0].T   # Re0
        Fo[:, :, 4] += f8p[1].T   # Re4
        Fo[:, :, 1] += f16p[0].T + 1j * f8p[2].T  # Re1 + i*Im1
        Fo[:, :, 2] += f16p[1].T + 1j * f8p[3].T  # Re2 + i*Im2
        Fo[:, :, 3] += f16p[2].T + 1j * f8p[4].T  # Re3 + i*Im3
        blk = np.fft.irfft(Fo, n=MINI, axis=-1).astype(np.float32)
        out[c * BS : (c + 1) * BS] = blk.reshape(BS, OUT_CH)
    return out


def _run(x: np.ndarray, eigens: np.ndarray, trace: bool = False):
    planes, wd = _host_pack(x, np.asarray(eigens, dtype=np.float32))
    nc = _build_nc()
    in_maps = [
        {
            "fx": np.ascontiguousarray(planes[:, :, i * BS : (i + 1) * BS]),
            "wd": wd,
        }
        for i in range(N_CORES)
    ]
    res = run_bass_kernel_spmd(nc, in_maps, list(range(N_CORES)), trace=trace)
    out = _host_unpack([res.results[i] for i in range(N_CORES)])
    return out, res


def kernel(x: np.ndarray, eigens: np.ndarray) -> np.ndarray:
    out, _ = _run(x, eigens)
    return out
